# revision 7
# baseline (speedup 1.0000x reference)
"""DocQA trilinear cross-attention kernel for 8 Trainium2 NeuronCores.

Sharding: data-parallel over batch (B=16 -> 2 batches per core). Params are
tiny and replicated; the two 1024-dim projections (x@w_input, key@w_key) and
the bf16 layout prep are folded into the host-side shard/unshard step.

Device computes, per batch b (XL=1024 x-rows, KL=512 key-rows, D=1024):
  ST[j,i] = sum_d keyT[d,j] * xdwT[d,i]          (S^T, d-contracted GEMM)
  eT[j,i] = exp(ST[j,i] + kl_eff[j])             (ACT exp, per-partition bias)
  x2k_raw[i,d] = sum_j eT[j,i] * key[j,d]        (GEMM; unnormalized)
  s1[jl,i] = sum_jc eT[jc*128+jl, i]  (fp32)     (DVE partial row-sum)
  m1[jl,i] = max_jc eT[jc*128+jl, i]  (bf16)     (DVE partial row-max)

The PE stream is pure N=512 GEMM work (128 matmuls/batch); exp lands in SBUF
directly (no transposes of e, no kl broadcast matmuls). Host finishes the
128-way partition fold of s1/m1, normalizes x2k_raw by 1/s, runs the tiny
key->x softmax + GEMV (0.1% of FLOPs), forms the elementwise output products
and the concat.
"""

import json

import numpy as np

import concourse.bass as bass
import concourse.tile as tile
from concourse import mybir

B, XL, KL, D = 16, 1024, 512, 1024
NCORES = 8
BPC = B // NCORES  # batches per core
NDC = D // 128     # d chunks (contraction)
NJC = KL // 128    # j chunks
NSI = XL // 512    # i super-tiles of 512
NEG = -10000000.0

FP = mybir.dt.float32
BF = mybir.dt.bfloat16


# --------------------------------------------------------------------------
# BIR post-pass: this container's walrus accepts only ONE sync-wait per
# instruction; Tile emits instructions carrying several. Hoist all but the
# last wait onto standalone single-wait EventSemaphore instructions placed
# immediately before (same engine queue => identical semantics).
# --------------------------------------------------------------------------
_bir_fix_installed = False


def _install_bir_fix():
    global _bir_fix_installed
    if _bir_fix_installed:
        return
    from concourse import bass2jax

    orig_compile = bass2jax.compile_bir_kernel

    def _split_multiwait_compile(bir_bytes, compile_dir, **kw):
        bir = json.loads(bir_bytes)
        n = 0
        for f in bir.get("functions", []):
            for blk in f.get("blocks", []):
                new_insts = []
                for ins in blk.get("instructions", []):
                    si = ins.get("sync_info") or {}
                    waits = si.get("on_wait") or []
                    if len(waits) > 1:
                        for w in waits[:-1]:
                            n += 1
                            new_insts.append({
                                "debug": ins.get("debug", 0),
                                "engine": ins["engine"],
                                "ins": [],
                                "outs": [],
                                "name": f"WSPL-{n}",
                                "opcode": "EventSemaphore",
                                "sync_info": {"on_update": [], "on_wait": [w]},
                            })
                        si["on_wait"] = [waits[-1]]
                    new_insts.append(ins)
                blk["instructions"] = new_insts
        return orig_compile(json.dumps(bir).encode(), compile_dir, **kw)

    bass2jax.compile_bir_kernel = _split_multiwait_compile
    _bir_fix_installed = True


# --------------------------------------------------------------------------
# Kernel program
# --------------------------------------------------------------------------
def build_nc(repeat: int = 1) -> bass.Bass:
    nc = bass.Bass()
    # host-prepared, p-major flattened layouts (see make_in_maps)
    xdwT_ext = nc.declare_dram_parameter("xdwT", [BPC, 128, NDC * XL], BF,
                                         isOutput=False)
    keyT_ext = nc.declare_dram_parameter("keyT", [BPC, 128, NDC * KL], BF,
                                         isOutput=False)
    key_ext = nc.declare_dram_parameter("key", [BPC, 128, NJC * D], BF,
                                        isOutput=False)
    klc_ext = nc.declare_dram_parameter("klc", [BPC, 128, NJC], FP,
                                        isOutput=False)
    x2k_ext = nc.declare_dram_parameter("x2k", [BPC, XL, D], BF, isOutput=True)
    s1_ext = nc.declare_dram_parameter("s1", [BPC, NSI, 128, 512], FP,
                                       isOutput=True)
    m1_ext = nc.declare_dram_parameter("m1", [BPC, NSI, 128, 512], BF,
                                       isOutput=True)

    with tile.TileContext(nc) as tc:
        from contextlib import ExitStack

        with ExitStack() as ctx:
            ep = ctx.enter_context  # shorthand

            inp = ep(tc.tile_pool(name="inp", bufs=3))
            epool = ep(tc.tile_pool(name="epool", bufs=2))
            red = ep(tc.tile_pool(name="red", bufs=2))
            stage = ep(tc.tile_pool(name="stage", bufs=3))

            ps_st = ep(tc.tile_pool(name="ps_st", bufs=2, space="PSUM"))
            ps_x2k = ep(tc.tile_pool(name="ps_x2k", bufs=3, space="PSUM"))

            def body():
                def emit_batch_loads(b):
                    # the sync ring carries ONLY loads: every store rides
                    # the ACT ring. Otherwise the next iteration's load kicks
                    # queue in the sync FIFO behind store kicks whose data
                    # isn't ready until late in this iteration, stalling the
                    # PE ~19us at every loop boundary. klc first (it gates
                    # every exp, hence PSUM recycling).
                    t = {}
                    klc = inp.tile([128, NJC], FP, tag="klc", name=f"klc{b}")
                    nc.sync.dma_start(klc[:], klc_ext[b])
                    t["klc"] = klc
                    keyT = inp.tile([128, NDC * KL], BF, tag="keyT",
                                    name=f"keyT{b}")
                    nc.sync.dma_start(keyT[:], keyT_ext[b])
                    t["keyT"] = keyT
                    xdwT = inp.tile([128, NDC * XL], BF, tag="xdwT",
                                    name=f"xdwT{b}")
                    nc.sync.dma_start(xdwT[:], xdwT_ext[b])
                    t["xdwT"] = xdwT
                    key = inp.tile([128, NJC * D], BF, tag="key", name=f"key{b}")
                    nc.sync.dma_start(key[:], key_ext[b])
                    t["key"] = key
                    return t

                tiles = emit_batch_loads(0)
                for b in range(BPC):
                    cur = tiles
                    xdwT, keyT, key, klc = (cur["xdwT"], cur["keyT"],
                                            cur["key"], cur["klc"])
                    eT = [[None] * NJC for _ in range(NSI)]

                    # ---- S^T GEMM + exp, per super-tile of 512 i ----
                    for si in range(NSI):
                        for jc in range(NJC):
                            ps = ps_st.tile([128, 512], FP, tag="st_ps")
                            for c in range(NDC):
                                nc.tensor.matmul(
                                    ps[:],
                                    keyT[:, c * KL + jc * 128:
                                         c * KL + (jc + 1) * 128],
                                    xdwT[:, c * XL + si * 512:
                                         c * XL + si * 512 + 512],
                                    start=(c == 0), stop=(c == NDC - 1),
                                )
                            e = epool.tile([128, 512], BF, tag=f"eT_{si}_{jc}",
                                           name=f"eT{b}_{si}_{jc}")
                            nc.scalar.activation(
                                e[:], ps[:], mybir.ActivationFunctionType.Exp,
                                bias=klc[:, jc:jc + 1],
                            )
                            eT[si][jc] = e

                    # hoist next batch loads ahead of this batch's stores
                    if b + 1 < BPC:
                        tiles = emit_batch_loads(b + 1)

                    for si in range(NSI):
                        e0, e1, e2, e3 = eT[si]
                        # ---- partial row-sum / row-max over jc (DVE) ----
                        sa = red.tile([128, 512], FP, tag="sa")
                        nc.vector.tensor_add(sa[:], e0[:], e1[:])
                        s1 = red.tile([128, 512], FP, tag="s1")
                        nc.vector.tensor_add(s1[:], e2[:], e3[:])
                        nc.vector.tensor_add(s1[:], s1[:], sa[:])
                        nc.scalar.dma_start(s1_ext[b, si], s1[:])
                        ma = red.tile([128, 512], BF, tag="ma")
                        nc.vector.tensor_max(ma[:], e0[:], e1[:])
                        m1 = red.tile([128, 512], BF, tag="m1")
                        nc.vector.tensor_max(m1[:], e2[:], e3[:])
                        nc.vector.tensor_max(m1[:], m1[:], ma[:])
                        nc.scalar.dma_start(m1_ext[b, si], m1[:])

                        # ---- x2k_raw GEMM per 128-row i chunk ----
                        for icl in range(4):
                            ic = si * 4 + icl
                            st = stage.tile([128, D], BF, tag="x2k_st")
                            for h in range(2):
                                px = ps_x2k.tile([128, 512], FP, tag="x2k_ps")
                                for jc in range(NJC):
                                    nc.tensor.matmul(
                                        px[:],
                                        eT[si][jc][:, icl * 128:
                                                   (icl + 1) * 128],
                                        key[:, jc * D + h * 512:
                                            jc * D + h * 512 + 512],
                                        start=(jc == 0), stop=(jc == NJC - 1),
                                    )
                                if h == 0:
                                    nc.scalar.activation(
                                        st[:, 0:512], px[:],
                                        mybir.ActivationFunctionType.Copy,
                                    )
                                else:
                                    nc.vector.tensor_copy(st[:, 512:1024],
                                                          px[:])
                            nc.scalar.dma_start(
                                x2k_ext[b, ic * 128:(ic + 1) * 128, :], st[:]
                            )

            if repeat == 1:
                body()
            else:
                with tc.For_i(0, repeat, 1):
                    body()

    return nc


# --------------------------------------------------------------------------
# Host entry point
# --------------------------------------------------------------------------
_cache = {}


def _get_nc(repeat: int = 1) -> bass.Bass:
    if repeat not in _cache:
        _cache[repeat] = build_nc(repeat)
    return _cache[repeat]


def make_in_maps(x, x_mask, key, key_mask, w_input, w_key, dot_w):
    import ml_dtypes

    x = np.asarray(x, np.float32)
    key = np.asarray(key, np.float32)
    key_mask = np.asarray(key_mask, np.float32)
    dot_w = np.asarray(dot_w, np.float32)
    w_key = np.asarray(w_key, np.float32)

    # (x * dot_w) transposed, p-major: [b, p, c*XL + i] = xdw[b, i, c*128+p]
    xdwT = np.ascontiguousarray(
        (x * dot_w).reshape(B, XL, NDC, 128).transpose(0, 3, 2, 1)
    ).reshape(B, 128, NDC * XL).astype(ml_dtypes.bfloat16)
    # key transposed (unscaled): [b, p, c*KL + j] = key[b, j, c*128+p]
    keyT = np.ascontiguousarray(
        key.reshape(B, KL, NDC, 128).transpose(0, 3, 2, 1)
    ).reshape(B, 128, NDC * KL).astype(ml_dtypes.bfloat16)
    # key natural, j-chunked: [b, p, jc*D + d] = key[b, jc*128+p, d]
    keyn = np.ascontiguousarray(
        key.reshape(B, NJC, 128, D).transpose(0, 2, 1, 3)
    ).reshape(B, 128, NJC * D).astype(ml_dtypes.bfloat16)
    # kl_eff column form: [b, p, jc] = kl_eff[b, jc*128+p]
    kl_eff = key @ w_key + (1.0 - key_mask) * NEG
    klc = np.ascontiguousarray(
        kl_eff.reshape(B, NJC, 128).transpose(0, 2, 1)
    ).astype(np.float32)

    in_maps = []
    for c in range(NCORES):
        s = slice(c * BPC, (c + 1) * BPC)
        in_maps.append({
            "xdwT": np.ascontiguousarray(xdwT[s]),
            "keyT": np.ascontiguousarray(keyT[s]),
            "key": np.ascontiguousarray(keyn[s]),
            "klc": np.ascontiguousarray(klc[s]),
        })
    return in_maps


def kernel(x, x_mask, key, key_mask, w_input, w_key, dot_w):
    from concourse.bass_utils import run_bass_kernel_spmd

    _install_bir_fix()
    nc = _get_nc(1)
    in_maps = make_in_maps(x, x_mask, key, key_mask, w_input, w_key, dot_w)
    res = run_bass_kernel_spmd(nc, in_maps, list(range(NCORES)))

    x = np.asarray(x, np.float32)
    x_mask = np.asarray(x_mask, np.float32)
    w_input = np.asarray(w_input, np.float32)

    x2k_raw = np.concatenate(
        [np.asarray(res.results[c]["x2k"]).astype(np.float32)
         for c in range(NCORES)], axis=0)                    # [B, XL, D]
    s1 = np.concatenate(
        [np.asarray(res.results[c]["s1"]).astype(np.float32)
         for c in range(NCORES)], axis=0)                    # [B, NSI, 128, 512]
    m1 = np.concatenate(
        [np.asarray(res.results[c]["m1"]).astype(np.float32)
         for c in range(NCORES)], axis=0)

    # fold the partition dim of the partial reductions
    s = s1.sum(axis=2).reshape(B, XL)                        # sum_j e
    mx = m1.max(axis=2).reshape(B, XL)                       # max_j e

    x2key = x2k_raw / s[:, :, None]

    # key -> x attention (tiny): max_s = xl + log max_j exp(kl + dot)
    xl = x @ w_input                                         # [B, XL]
    max_s = xl + np.log(mx)
    mxs = max_s * x_mask
    p = np.exp(mxs - mxs.max(axis=-1, keepdims=True))
    p = p / p.sum(axis=-1, keepdims=True)
    p = p * x_mask
    p = p / (p.sum(axis=-1, keepdims=True) + 1e-13)
    key2x = np.einsum("bx,bxd->bd", p.astype(np.float32), x)

    out = np.empty((B, XL, 4 * D), np.float32)
    out[..., 0:D] = x
    out[..., D:2 * D] = x2key
    out[..., 2 * D:3 * D] = x * x2key
    out[..., 3 * D:4 * D] = x * key2x[:, None, :]
    return out
